# revision 1
# baseline (speedup 1.0000x reference)
"""DINO loss kernel for Trainium2 (8 NeuronCores, Bass/Tile).

Math
----
Reference computes, with q = log_softmax(student/ts) [Ns=1280, D] and
p = softmax((teacher-center)/tt) [Nt=256, D]:

    loss = sum_{i != j} ( -sum_d p[i,d] q[j,d] ) / (Nt*Ns - Nt)

The full-pair sum factorizes over d:

    sum_{i,j} ce[i,j] = -sum_d P[d] * Q[d]
      P[d] = sum_i p[i,d]                (teacher prob column sums)
      Q[d] = sum_j q[j,d] = S[d]/ts - C  (S = raw student logit column sums,
                                          C = sum_j logsumexp_j(x/ts))
    diag  = sum_i sum_d p[i,d] q_g[i,d]
          = sum_i v_i/(ts*Z_i) - C_g     (v_i = sum_d e_t[i,d]*sg[i,d])

    loss = ( -(dot(P,S)/ts - C*sum(P)) + diag ) / (Nt*Ns - Nt)

So the device only does streaming reductions (no [Nt,Ns,D] einsum):
row sum-exp stats, raw column sums, teacher-prob column sums, and the
elementwise teacher*student_global dot for the diagonal.

Sharding (8 cores)
------------------
Pure data parallel over rows, one NEFF run, no collectives:
  core c gets student_local rows [128c,128c+128)           -> sl  [128, 65536]
           student_global rows [32c,32c+32) row-split x4   -> sg  [128, 16384]
           teacher rows        [32c,32c+32) row-split x4   -> t   [128, 16384]
Row-split x4: row i of a [32, 65536] slice is spread over partitions
4i..4i+3, 16384 columns each (a plain reshape(128, 16384) on the host),
so all engines run at full 128-partition width.

Implementation notes
--------------------
* Column sums run on the PE as mask-weighted matmuls in float32r (1 cyc/row
  vs 4 for fp32; requires every writer of a matmul operand to be f32r-typed,
  so the producing DMAs/activations write through f32r-bitcast APs).
* f32r matmuls only allow output partition base 0, so each PSUM tile is
  [32, 2048] holding 4 x [32, 512] regions side by side (rows 4..31 are
  zeros from the 32-wide masks); retired by one DVE copy + one [4, 2048]
  DMA per tile.
* Teacher softmax uses an exact on-device row max (cross-partition fold via
  two tiny DMAs). Student rows skip the device max pass: the exp bias is a
  host-sampled upper bound (sample max + margin) passed as input `nbs`;
  the host computes logsumexp against that same bound. If any resulting
  stat is non-finite (pathological input distribution), kernel() falls
  back to an exact numpy evaluation.
* All cross-core / cross-partition-group merging is float64 on the host.
"""

import numpy as np

import concourse.bass as bass
import concourse.bacc as bacc
import concourse.tile as tile
from concourse import mybir
from concourse.bass_utils import run_bass_kernel_spmd

F32 = mybir.dt.float32
F32R = mybir.dt.float32r
AX = mybir.AxisListType
EXP = mybir.ActivationFunctionType.Exp

N_CORES = 8
D = 65536
N_T = 256
N_G = 256
N_L = 1024
SL_ROWS = N_L // N_CORES          # 128 student_local rows per core
SG_ROWS = N_G // N_CORES          # 32 student_global rows per core
T_ROWS = N_T // N_CORES           # 32 teacher rows per core


def _masks(P=128):
    # M=32 masks: matmul output covers a full 32-row block so the PSUM
    # region is fully written (rows past the 4 real ones get zeros).
    # qmask[p, m] = 1 if m == p % 4   (row-split quarter column sums)
    qmask = np.zeros((P, 32), np.float32)
    qmask[np.arange(P), np.arange(P) % 4] = 1.0
    # emask block q ([:, 32q:32q+32]) has ones only in column q: lhsT that
    # adds a plain colsum into row q of a 32-row PSUM region.
    emask = np.zeros((P, 128), np.float32)
    for q in range(4):
        emask[:, 32 * q + q] = 1.0
    return qmask, emask


def build_nc(D=D, n_sl_chunks=16, ts=0.1, tt=0.04):
    """Build the per-core Bass program. All 8 cores run this same NEFF."""
    DQ = D // 4                    # columns per quarter
    CQ = DQ // n_sl_chunks         # sl chunk columns per quarter
    reg = 512                      # matmul free size (one PSUM bank)
    assert CQ % reg == 0
    rpc = CQ // reg                # regions per sl chunk
    bank_n = 2 * reg               # quarter-cols per PSUM tile [32, bank_n]
    assert DQ % bank_n == 0
    cpt = bank_n // CQ             # sl chunks per psum tile
    cht = DQ // 4                  # teacher/sg activation chunk size

    nc = bacc.Bacc()
    sl = nc.dram_tensor("sl", [128, D], F32, kind="ExternalInput")
    sg = nc.dram_tensor("sg", [128, DQ], F32, kind="ExternalInput")
    t = nc.dram_tensor("t", [128, DQ], F32, kind="ExternalInput")
    nbs = nc.dram_tensor("nbs", [128, 1], F32, kind="ExternalInput")

    qmask_np, emask_np = _masks()
    qmask_d = nc.inline_tensor(qmask_np, name="qmask_c")
    emask_d = nc.inline_tensor(emask_np, name="emask_c")

    s_sl = nc.dram_tensor("s_sl", [4, DQ], F32, kind="ExternalOutput")
    s_sg = nc.dram_tensor("s_sg", [4, DQ], F32, kind="ExternalOutput")
    p_out = nc.dram_tensor("p_out", [4, DQ], F32, kind="ExternalOutput")
    w_sl = nc.dram_tensor("w_sl", [128, n_sl_chunks], F32, kind="ExternalOutput")
    w_sg = nc.dram_tensor("w_sg", [128, 4], F32, kind="ExternalOutput")
    z_t = nc.dram_tensor("z_t", [128, 4], F32, kind="ExternalOutput")
    v_t = nc.dram_tensor("v_t", [128, 4], F32, kind="ExternalOutput")

    with tile.TileContext(nc) as tc:
        with (
            tc.tile_pool(name="singles", bufs=1) as singles,
            tc.tile_pool(name="big", bufs=1) as big,
            tc.tile_pool(name="chunks", bufs=3) as chunks,
            tc.tile_pool(name="escr", bufs=1) as escr,
            tc.tile_pool(name="stats", bufs=1) as stats,
            tc.tile_pool(name="stage", bufs=2) as stage_pool,
            tc.tile_pool(name="psA", bufs=2, space="PSUM") as psA,
            tc.tile_pool(name="psB", bufs=2, space="PSUM") as psB,
        ):
            # NOTE: each engine executes its instructions in emission order,
            # so this body is laid out in expected readiness order, not by
            # logical phase: teacher chain first (it gates wq -> P), then
            # student_global, then the long student_local stream.
            qmask = singles.tile([128, 32], F32)
            nc.sync.dma_start(out=qmask.bitcast(F32R), in_=qmask_d[:, :].bitcast(F32R))
            emask = singles.tile([128, 128], F32)
            nc.sync.dma_start(out=emask.bitcast(F32R), in_=emask_d[:, :].bitcast(F32R))
            nbs_t = singles.tile([128, 1], F32)
            nc.sync.dma_start(out=nbs_t, in_=nbs[:, :])

            tr = big.tile([128, DQ], F32)
            sgr = big.tile([128, DQ], F32)
            mT4 = stats.tile([128, 4], F32)
            for j in range(4):
                nc.sync.dma_start(
                    out=tr[:, j * cht : (j + 1) * cht].bitcast(F32R),
                    in_=t[:, j * cht : (j + 1) * cht].bitcast(F32R),
                )
                nc.vector.reduce_max(
                    mT4[:, j : j + 1], tr[:, j * cht : (j + 1) * cht], axis=AX.X
                )
            for j in range(4):
                nc.sync.dma_start(
                    out=sgr[:, j * cht : (j + 1) * cht].bitcast(F32R),
                    in_=sg[:, j * cht : (j + 1) * cht].bitcast(F32R),
                )

            wS = stats.tile([128, n_sl_chunks], F32)

            def retire(stpool, bank, dst, bank_i, on_act=False):
                """PSUM [32, bank_n] -> SBUF -> one [4, bank_n] DMA."""
                st = stpool.tile([32, bank_n], F32, tag="stage")
                if on_act:
                    nc.scalar.activation(st, bank,
                                         mybir.ActivationFunctionType.Copy)
                else:
                    nc.vector.tensor_copy(out=st, in_=bank)
                nc.sync.dma_start(
                    out=dst[:, bank_i * bank_n : (bank_i + 1) * bank_n],
                    in_=st[0:4, :],
                )

            # teacher row max (exact): fold partials, broadcast per row
            mT = stats.tile([128, 1], F32)
            nc.vector.reduce_max(mT, mT4, axis=AX.X)
            tp4 = stats.tile([32, 4], F32)
            nc.sync.dma_start(out=tp4, in_=mT)
            mrow = stats.tile([32, 1], F32)
            nc.vector.reduce_max(mrow, tp4, axis=AX.X)
            mb = stats.tile([128, 1], F32)
            nc.sync.dma_start(
                out=mb,
                in_=bass.AP(tensor=mrow.tensor, offset=mrow.offset,
                            ap=[[1, 32], [0, 4]]),
            )
            nmb = stats.tile([128, 1], F32)
            nc.vector.tensor_scalar_mul(nmb, mb, -1.0 / tt)
            # teacher exp (in-place, f32r) + row partial sums
            zT = stats.tile([128, 4], F32)
            for j in range(4):
                nc.scalar.activation(
                    tr[:, j * cht : (j + 1) * cht].bitcast(F32R),
                    tr[:, j * cht : (j + 1) * cht],
                    EXP, bias=nmb, scale=1.0 / tt,
                    accum_out=zT[:, j : j + 1],
                )
            # Z fold + 1/Z-weighted mask for P
            zloc = stats.tile([128, 1], F32)
            nc.vector.reduce_sum(zloc, zT, axis=AX.X)
            tz4 = stats.tile([32, 4], F32)
            nc.sync.dma_start(out=tz4, in_=zloc)
            z32 = stats.tile([32, 1], F32)
            nc.vector.reduce_sum(z32, tz4, axis=AX.X)
            rz32 = stats.tile([32, 1], F32)
            nc.vector.reciprocal(rz32, z32)
            rzb = stats.tile([128, 1], F32)
            nc.sync.dma_start(
                out=rzb,
                in_=bass.AP(tensor=rz32.tensor, offset=rz32.offset,
                            ap=[[1, 32], [0, 4]]),
            )
            wq = stats.tile([128, 32], F32)
            nc.vector.tensor_scalar_mul(wq.bitcast(F32R), qmask, rzb)

            # student_global exp stats (scratch out; sgr stays raw; same
            # host-supplied bound as student_local). Emitted one at a time,
            # woven into the schedule below (ACT executes in emission order).
            wG = stats.tile([128, 4], F32)

            def sg_exp(j):
                sc = escr.tile([128, cht], F32, tag="escr")
                nc.scalar.activation(
                    sc, sgr[:, j * cht : (j + 1) * cht],
                    EXP, bias=nbs_t, scale=1.0 / ts,
                    accum_out=wG[:, j : j + 1],
                )

            def _abank(lhsT, srct, dst, bank_i):
                bank = psA.tile([32, bank_n], F32, tag="bankA")
                for s in range(bank_n // reg):
                    lo = bank_i * bank_n + s * reg
                    nc.tensor.matmul(
                        bank[:, s * reg : (s + 1) * reg],
                        lhsT.bitcast(F32R),
                        srct[:, lo : lo + reg].bitcast(F32R),
                        start=True, stop=True,
                    )
                retire(stage_pool, bank, dst, bank_i)

            def sg_bank(bank_i):
                _abank(qmask, sgr, s_sg, bank_i)

            def p_bank(bank_i):
                _abank(wq, tr, p_out, bank_i)

            # v_hat: in-place multiply over exp'd teacher + row-sum, both on
            # DVE (gpsimd shares the SBUF port with DVE and is 2x slower;
            # tensor_tensor_reduce dies at runtime on this stack). The P
            # banks for quarter j must be emitted before vhat(j).
            vT = stats.tile([128, 4], F32)

            def vhat(j):
                nc.vector.tensor_mul(
                    tr[:, j * cht : (j + 1) * cht].bitcast(F32R),
                    tr[:, j * cht : (j + 1) * cht],
                    sgr[:, j * cht : (j + 1) * cht],
                )
                nc.vector.reduce_sum(vT[:, j : j + 1],
                                     tr[:, j * cht : (j + 1) * cht], axis=AX.X)

            # student_local bank: chunk DMA, colsum matmuls, exp on ACT
            # (exp and matmuls both read the raw chunk; exp writes a
            # throwaway scratch so they don't serialize); retire on ACT so
            # the stream has no DVE dependency at all.
            slv = sl.rearrange("p (q k c) -> p q k c", q=4, k=n_sl_chunks)

            def sl_bank(bank_i):
                bank = psB.tile([32, bank_n], F32, tag="bankB")
                for kk in range(cpt):
                    k = bank_i * cpt + kk
                    ch = chunks.tile([128, 4, CQ], F32, tag="chunk")
                    nc.sync.dma_start(
                        out=ch.bitcast(F32R), in_=slv[:, :, k, :].bitcast(F32R)
                    )
                    for s in range(rpc):
                        rl = kk * rpc + s
                        for q in range(4):
                            nc.tensor.matmul(
                                bank[:, rl * reg : (rl + 1) * reg],
                                emask[:, 32 * q : 32 * q + 32].bitcast(F32R),
                                ch[:, q, s * reg : (s + 1) * reg].bitcast(F32R),
                                start=(q == 0),
                                stop=(q == 3),
                            )
                    sc = escr.tile([128, 4 * CQ], F32, tag="escr")
                    nc.scalar.activation(
                        sc.rearrange("p (q c) -> p q c", q=4), ch, EXP,
                        bias=nbs_t, scale=1.0 / ts,
                        accum_out=wS[:, k : k + 1],
                    )
                retire(stage_pool, bank, s_sl, bank_i, on_act=True)

            # ---- interleaved schedule (per-engine order == emission) ----
            nb = DQ // bank_n
            if nb >= 16:
                sg_exp(0)
                for i in range(nb):
                    sg_bank(i)
                for i in range(0, 4):
                    sl_bank(i)
                sg_exp(1)
                for i in range(4, 6):
                    sl_bank(i)
                for i in range(0, nb // 2):
                    p_bank(i)
                vhat(0)
                vhat(1)
                for i in range(6, 8):
                    sl_bank(i)
                sg_exp(2)
                for i in range(nb // 2, nb):
                    p_bank(i)
                vhat(2)
                vhat(3)
                for i in range(8, 12):
                    sl_bank(i)
                sg_exp(3)
                for i in range(12, nb):
                    sl_bank(i)
            else:
                for j in range(4):
                    sg_exp(j)
                for i in range(nb):
                    sg_bank(i)
                for i in range(nb):
                    p_bank(i)
                for j in range(4):
                    vhat(j)
                for i in range(nb):
                    sl_bank(i)

            nc.sync.dma_start(out=w_sl[:, :], in_=wS)
            nc.sync.dma_start(out=w_sg[:, :], in_=wG)
            nc.sync.dma_start(out=z_t[:, :], in_=zT)
            nc.sync.dma_start(out=v_t[:, :], in_=vT)

    nc.compile()
    return nc


_NC_CACHE = {}


def _get_nc(ts, tt):
    key = (round(ts, 9), round(tt, 9))
    if key not in _NC_CACHE:
        _NC_CACHE[key] = build_nc(ts=ts, tt=tt)
    return _NC_CACHE[key]


def _merge(results, ts, tt, bs_scaled):
    """Host-side exact merge of per-core device outputs (float64).

    bs_scaled = b_s/ts, the (already scaled) exp bound the device used for
    student_local rows. Returns (loss, healthy).
    """
    S = np.zeros(D, np.float64)
    P = np.zeros(D, np.float64)
    C = 0.0       # sum of all student row logsumexps
    C_g = 0.0     # global-student-row portion
    diag1 = 0.0   # sum_i v_i / (ts * Z_i)
    healthy = True
    for r in results:
        S += r["s_sl"].astype(np.float64).reshape(-1)
        S += r["s_sg"].astype(np.float64).reshape(-1)
        P += r["p_out"].astype(np.float64).reshape(-1)
        # student_local rows: common bound -> lse = b/ts + log(sum w)
        w = r["w_sl"].astype(np.float64)               # [128, nch]
        wsum = w.sum(axis=1)
        healthy &= bool(np.isfinite(w).all() and (wsum > 0).all())
        C += (bs_scaled + np.log(np.maximum(wsum, 1e-300))).sum()
        # student_global rows: common bound per-partition lse -> merge 4s
        wg = r["w_sg"].astype(np.float64).sum(axis=1)  # [128]
        healthy &= bool(np.isfinite(wg).all() and (wg > 0).all())
        lp = (bs_scaled + np.log(np.maximum(wg, 1e-300))).reshape(32, 4)
        mxg = lp.max(axis=1, keepdims=True)
        lse_g = mxg[:, 0] + np.log(np.exp(lp - mxg).sum(axis=1))
        C += lse_g.sum()
        C_g += lse_g.sum()
        # teacher diagonal: v_i / Z_i (common per-row exp offset cancels)
        v = r["v_t"].astype(np.float64).sum(axis=1).reshape(32, 4).sum(axis=1)
        z = r["z_t"].astype(np.float64).sum(axis=1).reshape(32, 4).sum(axis=1)
        healthy &= bool(np.isfinite(v).all() and np.isfinite(z).all()
                        and (z > 0).all())
        diag1 += (v / np.maximum(z, 1e-300)).sum() / ts
        healthy &= bool(np.isfinite(r["s_sl"]).all()
                        and np.isfinite(r["s_sg"]).all()
                        and np.isfinite(r["p_out"]).all())

    cross = P @ S / ts - C * P.sum()
    diag = diag1 - C_g
    total = -cross + diag
    n_s = N_G + N_L
    n_loss_terms = N_T * n_s - min(N_T, n_s)
    loss = total / n_loss_terms
    healthy &= bool(np.isfinite(loss))
    return loss, healthy


def _numpy_loss(sg_full, sl_full, teacher, ts, tt):
    """Exact host fallback (never hit for sane input distributions)."""
    x = np.concatenate([sg_full, sl_full], axis=0).astype(np.float64) / ts
    lq = x - x.max(axis=1, keepdims=True)
    lq -= np.log(np.exp(lq).sum(axis=1, keepdims=True))
    y = teacher.astype(np.float64) / tt
    e = np.exp(y - y.max(axis=1, keepdims=True))
    p = e / e.sum(axis=1, keepdims=True)
    ce = -(p @ lq.T)
    n_t, n_s = ce.shape
    idx = np.arange(n_t)
    ce[idx, idx] = 0.0
    return ce.sum() / (n_t * n_s - min(n_t, n_s))


def kernel(out_student_global, out_student_local, out_teacher, center,
           temp_student, temp_teacher, cent_rate_m):
    out_student_global = np.asarray(out_student_global)
    out_student_local = np.asarray(out_student_local)
    out_teacher = np.asarray(out_teacher)
    center = np.asarray(center)
    ts = float(np.asarray(temp_student).reshape(-1)[0])
    tt = float(np.asarray(temp_teacher).reshape(-1)[0])

    teacher = out_teacher
    if np.any(center):
        teacher = out_teacher - center.reshape(1, -1).astype(np.float32)
    teacher = np.ascontiguousarray(teacher, dtype=np.float32)
    sg_full = np.ascontiguousarray(out_student_global, dtype=np.float32)
    sl_full = np.ascontiguousarray(out_student_local, dtype=np.float32)

    # Safe exp bound for student rows: strided-sample max + margin.
    smax = max(float(sl_full.ravel()[::257].max()),
               float(sg_full.ravel()[::257].max()))
    b_s = smax + 1.0
    nbs = np.full((128, 1), -b_s / ts, np.float32)

    nc = _get_nc(ts, tt)
    in_maps = []
    for c in range(N_CORES):
        in_maps.append({
            "sl": sl_full[c * SL_ROWS:(c + 1) * SL_ROWS],
            "sg": sg_full[c * SG_ROWS:(c + 1) * SG_ROWS].reshape(128, D // 4),
            "t": teacher[c * T_ROWS:(c + 1) * T_ROWS].reshape(128, D // 4),
            "nbs": nbs,
        })
    res = run_bass_kernel_spmd(nc, in_maps, core_ids=list(range(N_CORES)))
    loss, healthy = _merge(res.results, ts, tt, b_s / ts)
    if not healthy:
        loss = _numpy_loss(sg_full, sl_full, teacher, ts, tt)
    return np.float32(loss)



# revision 12
# speedup vs baseline: 2.1480x; 2.1480x over previous
"""DINO loss kernel for Trainium2 (8 NeuronCores, Bass/Tile).

Math
----
Reference computes, with q = log_softmax(student/ts) [Ns=1280, D] and
p = softmax((teacher-center)/tt) [Nt=256, D]:

    loss = sum_{i != j} ( -sum_d p[i,d] q[j,d] ) / (Nt*Ns - Nt)

The full-pair sum factorizes over d:

    sum_{i,j} ce[i,j] = -sum_d P[d] * Q[d]
      P[d] = sum_i p[i,d]                (teacher prob column sums)
      Q[d] = sum_j q[j,d] = S[d]/ts - C  (S = raw student logit column sums,
                                          C = sum_j logsumexp_j(x/ts))
    diag  = sum_i sum_d p[i,d] q_g[i,d]
          = sum_i v_i/(ts*Z_i) - C_g     (v_i = sum_d e_t[i,d]*sg[i,d])

    loss = ( -(dot(P,S)/ts - C*sum(P)) + diag ) / (Nt*Ns - Nt)

So the device only does streaming reductions (no [Nt,Ns,D] einsum):
row sum-exp stats, raw column sums, teacher-prob column sums, and the
elementwise teacher*student_global dot for the diagonal.

Sharding (8 cores)
------------------
Pure data parallel over rows, one NEFF run, no collectives:
  core c gets student_local rows [128c,128c+128)           -> sl  [128, 65536]
           student_global rows [32c,32c+32) row-split x4   -> sg  [128, 16384]
           teacher rows        [32c,32c+32) row-split x4   -> t   [128, 16384]
Row-split x4: row i of a [32, 65536] slice is spread over partitions
4i..4i+3, 16384 columns each (a plain reshape(128, 16384) on the host),
so all engines run at full 128-partition width.

Implementation notes
--------------------
* All big inputs travel as bf16 (half the HBM/DMA traffic; quantization
  error on the final scalar is ~2e-5, tolerance is 2e-2). The teacher is
  exp'd in place in bf16; p normalization (e/Z) uses consistently rounded
  values so ratios stay exact to bf16 element precision.
* Both student AND teacher exps use host-sampled upper bounds (sample max
  + margin) as the common exp bias. Softmax ratios are shift-invariant,
  so the bound only has to avoid overflow (margin keeps x - b well below
  88*tt) -- elements far below the bound underflow to 0 and contribute
  ~e^-60 relative, i.e. nothing. This removes the whole on-device teacher
  row-max fold chain. If any stat comes back non-finite (pathological
  input distribution), kernel() falls back to an exact numpy evaluation.
* Column sums run on the PE as mask-weighted bf16 matmuls (1 cyc/row).
  Bank tiles are [4, 1024] in PSUM (exactly the real output rows).
* DMA queue discipline (the big scheduling win): the SP/sync queue
  carries ONLY input loads, in consumption order, so no compute-dependent
  DMA ever stalls a load (a waiting DMA holds its engine's sequencer).
  PSUM retires for the student_local stream go PSUM->DRAM directly on the
  Pool/SWDGE queue; sg/p banks stage through SBUF on DVE (fast PSUM WAR
  release so the PE never stalls on bank reuse) and then DRAM-out on Pool.
* All cross-core / cross-partition-group merging is float64 on the host.
"""

import numpy as np
import ml_dtypes

import concourse.bass as bass
import concourse.bacc as bacc
import concourse.tile as tile
from concourse import mybir
from concourse.bass_utils import run_bass_kernel_spmd

F32 = mybir.dt.float32
BF16 = mybir.dt.bfloat16
AX = mybir.AxisListType
EXP = mybir.ActivationFunctionType.Exp
BF = ml_dtypes.bfloat16

N_CORES = 8
D = 65536
N_T = 256
N_G = 256
N_L = 1024
SL_ROWS = N_L // N_CORES          # 128 student_local rows per core
SG_ROWS = N_G // N_CORES          # 32 student_global rows per core
T_ROWS = N_T // N_CORES           # 32 teacher rows per core


def _masks(P=128):
    # qmask[p, m] = 1 if m == p % 4: out row m = colsum over partition group
    # m (row-split x4 quarter colsums, weighted later by 1/Z for P).
    qmask = np.zeros((P, 4), BF)
    qmask[np.arange(P), np.arange(P) % 4] = 1.0
    # emask block q ([:, 4q:4q+4]) has ones only in column q: lhsT that adds
    # a plain colsum of quarter q into row q of a 4-row PSUM region.
    emask = np.zeros((P, 16), BF)
    for q in range(4):
        emask[:, 4 * q + q] = 1.0
    # gmask[p', p] = 1 iff p'//4 == p//4: one matmul folds the 4 per-quarter
    # partial Z's of each logical row and broadcasts to all 4 partitions.
    gmask = np.zeros((P, P), BF)
    for r in range(P // 4):
        gmask[4 * r : 4 * r + 4, 4 * r : 4 * r + 4] = 1.0
    return qmask, emask, gmask


def build_nc(D=D, n_sl_chunks=8, ts=0.1, tt=0.04):
    """Build the per-core Bass program. All 8 cores run this same NEFF."""
    DQ = D // 4                    # 16384 columns per quarter
    CQ = DQ // n_sl_chunks         # 2048 sl quarter-cols per DMA chunk
    reg = 512                      # matmul free size (one PSUM bank of f32)
    bank_n = 2 * reg               # quarter-cols per PSUM bank tile
    assert CQ % bank_n == 0
    bpc = CQ // bank_n             # sl bank tiles per chunk
    nb = DQ // bank_n              # bank tiles per full stream (sg/p)
    cht = DQ // 4                  # teacher/sg DMA+exp chunk size

    nc = bacc.Bacc()
    sl = nc.dram_tensor("sl", [128, D], BF16, kind="ExternalInput")
    sg = nc.dram_tensor("sg", [128, DQ], BF16, kind="ExternalInput")
    t = nc.dram_tensor("t", [128, DQ], BF16, kind="ExternalInput")
    nbs = nc.dram_tensor("nbs", [128, 1], F32, kind="ExternalInput")
    ntb = nc.dram_tensor("ntb", [128, 1], F32, kind="ExternalInput")

    qmask_np, emask_np, gmask_np = _masks()
    qmask_d = nc.inline_tensor(qmask_np, name="qmask_c")
    emask_d = nc.inline_tensor(emask_np, name="emask_c")
    gmask_d = nc.inline_tensor(gmask_np, name="gmask_c")

    s_sl = nc.dram_tensor("s_sl", [4, DQ], F32, kind="ExternalOutput")
    s_sg = nc.dram_tensor("s_sg", [4, DQ], F32, kind="ExternalOutput")
    p_out = nc.dram_tensor("p_out", [4, DQ], F32, kind="ExternalOutput")
    w_sl = nc.dram_tensor("w_sl", [128, n_sl_chunks], F32, kind="ExternalOutput")
    w_sg = nc.dram_tensor("w_sg", [128, 4], F32, kind="ExternalOutput")

    with tile.TileContext(nc) as tc:
        with (
            tc.tile_pool(name="singles", bufs=1) as singles,
            tc.tile_pool(name="big", bufs=1) as big,
            tc.tile_pool(name="chunks", bufs=3) as chunks,
            tc.tile_pool(name="escr", bufs=1) as escr,
            tc.tile_pool(name="stats", bufs=1) as stats,
            tc.tile_pool(name="stage", bufs=4) as stage_pool,
            tc.tile_pool(name="psA", bufs=2, space="PSUM") as psA,
            tc.tile_pool(name="psB", bufs=3, space="PSUM") as psB,
            tc.tile_pool(name="psC", bufs=1, space="PSUM") as psC,
        ):
            # The SP/sync queue carries ONLY input loads, in consumption
            # order. Nothing on it ever waits on compute.
            tr = big.tile([128, DQ], BF16)
            sgr = big.tile([128, DQ], BF16)
            # Teacher arrives in 5 pieces -- a small first piece so the ACT
            # exp stream starts as early as possible.
            tch = [(0, 2048), (2048, 2048), (4096, 4096),
                   (8192, 4096), (12288, 4096)]
            nbs_t = singles.tile([128, 1], F32)
            nc.sync.dma_start(out=nbs_t, in_=nbs[:, :])
            ntb_t = singles.tile([128, 1], F32)
            nc.sync.dma_start(out=ntb_t, in_=ntb[:, :])
            qmask = singles.tile([128, 4], BF16)
            nc.sync.dma_start(out=qmask, in_=qmask_d[:, :])
            emask = singles.tile([128, 16], BF16)
            nc.sync.dma_start(out=emask, in_=emask_d[:, :])
            gmask = singles.tile([128, 128], BF16)
            nc.sync.dma_start(out=gmask, in_=gmask_d[:, :])
            nc.sync.dma_start(out=tr[:, 0:2048], in_=t[:, 0:2048])
            for lo, n in tch[1:]:
                nc.sync.dma_start(out=tr[:, lo : lo + n], in_=t[:, lo : lo + n])
            for j in range(4):
                nc.sync.dma_start(
                    out=sgr[:, j * cht : (j + 1) * cht],
                    in_=sg[:, j * cht : (j + 1) * cht],
                )

            # Warm the ACT exp table while the first teacher piece loads.
            warm = stats.tile([128, 1], F32)
            nc.vector.memset(warm, 0.0)
            nc.scalar.activation(warm, warm, EXP)

            # teacher exp (in place, bf16) + per-partition partial Z sums
            zT = stats.tile([128, len(tch)], F32)
            for j, (lo, n) in enumerate(tch):
                nc.scalar.activation(
                    tr[:, lo : lo + n],
                    tr[:, lo : lo + n],
                    EXP, bias=ntb_t, scale=1.0 / tt,
                    accum_out=zT[:, j : j + 1],
                )

            # Z fold across the 4 partitions of each logical teacher row:
            # one block-diagonal matmul broadcasts the group sums back to
            # every partition (NO DMA -- a tiny compute-dependent DMA here
            # would queue behind all prefetched loads on the DMA engines).
            zloc = stats.tile([128, 1], BF16)
            with nc.allow_low_precision(reason="Z fold feeds a bf16 matmul; "
                                        "0.4% on Z is far inside tolerance"):
                nc.vector.reduce_sum(zloc, zT, axis=AX.X)
            zfold = psC.tile([128, 1], F32)
            nc.tensor.matmul(zfold, gmask, zloc, start=True, stop=True)
            rzb = stats.tile([128, 1], F32)
            nc.vector.reciprocal(rzb, zfold)
            wq = stats.tile([128, 4], BF16)
            nc.vector.tensor_scalar_mul(wq, qmask, rzb)

            # student_global exp stats (scratch out; sgr stays raw; same
            # host-supplied bound as student_local).
            wG = stats.tile([128, 4], F32)

            def sg_exp(j):
                sc = escr.tile([128, cht], BF16, tag="escr")
                nc.scalar.activation(
                    sc, sgr[:, j * cht : (j + 1) * cht],
                    EXP, bias=nbs_t, scale=1.0 / ts,
                    accum_out=wG[:, j : j + 1],
                )

            # PSUM retire: bank -> SBUF stage slot on DVE (fast WAR release
            # so the PE never waits on a DMA round-trip); one Pool/SWDGE DMA
            # stores each filled [4, 4096] stage tile. Big stage groups +
            # deep buffering so store transfers queueing behind prefetched
            # loads on the DMA engines never back-propagates to the PE.
            def make_stream(dst, bank_cols, group):
                state = {"st": None, "n0": 0, "cnt": 0}

                def add(bank, bank_i):
                    if state["st"] is None:
                        st = stage_pool.tile(
                            [4, group * bank_cols], F32, tag="stage",
                            name="st")
                        state["st"] = st
                        state["n0"] = bank_i
                        state["cnt"] = 0
                    q = state["cnt"]
                    nc.vector.tensor_copy(
                        out=state["st"][:, q * bank_cols : (q + 1) * bank_cols],
                        in_=bank)
                    state["cnt"] += 1
                    if state["cnt"] == group:
                        nc.gpsimd.dma_start(
                            out=dst[:, state["n0"] * bank_cols
                                    : (state["n0"] + group) * bank_cols],
                            in_=state["st"])
                        state["st"] = None
                return add

            sg_ret = make_stream(s_sg, bank_n, 4)
            p_ret = make_stream(p_out, bank_n, 4)
            sl_ret = make_stream(s_sl, reg, 8)

            def _abank(lhsT, srct, ret, bank_i):
                bank = psA.tile([4, bank_n], F32, tag="bankA")
                for s in range(bank_n // reg):
                    lo = bank_i * bank_n + s * reg
                    nc.tensor.matmul(
                        bank[:, s * reg : (s + 1) * reg],
                        lhsT,
                        srct[:, lo : lo + reg],
                        start=True, stop=True,
                    )
                ret(bank, bank_i)

            def sg_bank(bank_i):
                _abank(qmask, sgr, sg_ret, bank_i)

            def p_bank(bank_i):
                _abank(wq, tr, p_ret, bank_i)

            # student_local stream: chunk DMA (sync queue), colsum matmuls
            # (PE, [4, reg] banks), exp on ACT (throwaway scratch + row-sum
            # accumulator), retire via the shared stage/store path.
            slv = sl.rearrange("p (q k c) -> p q k c", q=4, k=n_sl_chunks)
            wS = stats.tile([128, n_sl_chunks], F32)

            def sl_chunk(k):
                ch = chunks.tile([128, 4, CQ], BF16, tag="chunk")
                nc.sync.dma_start(out=ch, in_=slv[:, :, k, :])
                sc = escr.tile([128, 4 * CQ], BF16, tag="escr")
                nc.scalar.activation(
                    sc.rearrange("p (q c) -> p q c", q=4), ch, EXP,
                    bias=nbs_t, scale=1.0 / ts,
                    accum_out=wS[:, k : k + 1],
                )
                for b in range(CQ // reg):
                    bank = psB.tile([4, reg], F32, tag="bankB")
                    cl = b * reg
                    for q in range(4):
                        nc.tensor.matmul(
                            bank,
                            emask[:, 4 * q : 4 * q + 4],
                            ch[:, q, cl : cl + reg],
                            start=(q == 0),
                            stop=(q == 3),
                        )
                    sl_ret(bank, k * (CQ // reg) + b)

            # ---- interleaved schedule (per-engine order == emission; the
            # tile scheduler may refine within dependency limits) ----
            for j in range(4):
                sg_exp(j)
            for i in range(nb):
                sg_bank(i)
            sl_chunk(0)
            for i in range(0, 4):
                p_bank(i)
            sl_chunk(1)
            for i in range(4, 8):
                p_bank(i)
            sl_chunk(2)
            for i in range(8, 12):
                p_bank(i)
            sl_chunk(3)
            for i in range(12, 16):
                p_bank(i)
            for k in range(4, n_sl_chunks):
                sl_chunk(k)

            # stats out: w_sg rides the Pool queue (ready mid-stream); w_sl
            # (gated by the final sl exp) goes at the very end of ACT's own
            # queue so it cannot head-of-line-block the Pool stores.
            nc.gpsimd.dma_start(out=w_sg[:, :], in_=wG)
            nc.scalar.dma_start(out=w_sl[:, :], in_=wS)

    nc.compile()
    return nc


_NC_CACHE = {}


def _get_nc(ts, tt):
    key = (round(ts, 9), round(tt, 9))
    if key not in _NC_CACHE:
        _NC_CACHE[key] = build_nc(ts=ts, tt=tt)
    return _NC_CACHE[key]


def _merge(results, ts, tt, bs_scaled, diag1, n_sl_chunks=8):
    """Host-side exact merge of per-core device outputs (float64).

    bs_scaled = b_s/ts, the (already scaled) exp bound the device used for
    student rows; diag1 = sum_i v_i/(ts*Z_i), computed exactly on the host
    (O(Nt*D), ~0.1% of the kernel flops). Returns (loss, healthy).
    """
    S = np.zeros(D, np.float64)
    P = np.zeros(D, np.float64)
    C = 0.0       # sum of all student row logsumexps
    C_g = 0.0     # global-student-row portion
    healthy = True
    for r in results:
        S += r["s_sl"].astype(np.float64).reshape(-1)
        S += r["s_sg"].astype(np.float64).reshape(-1)
        P += r["p_out"].astype(np.float64).reshape(-1)
        # student_local rows: common bound -> lse = b/ts + log(sum w)
        w = r["w_sl"].astype(np.float64)               # [128, nch]
        wsum = w.sum(axis=1)
        healthy &= bool(np.isfinite(w).all() and (wsum > 0).all())
        C += (bs_scaled + np.log(np.maximum(wsum, 1e-300))).sum()
        # student_global rows: common bound per-partition lse -> merge 4s
        wg = r["w_sg"].astype(np.float64).sum(axis=1)  # [128]
        healthy &= bool(np.isfinite(wg).all() and (wg > 0).all())
        lp = (bs_scaled + np.log(np.maximum(wg, 1e-300))).reshape(32, 4)
        mxg = lp.max(axis=1, keepdims=True)
        lse_g = mxg[:, 0] + np.log(np.exp(lp - mxg).sum(axis=1))
        C += lse_g.sum()
        C_g += lse_g.sum()
        healthy &= bool(np.isfinite(r["s_sl"]).all()
                        and np.isfinite(r["s_sg"]).all()
                        and np.isfinite(r["p_out"]).all())

    cross = P @ S / ts - C * P.sum()
    diag = diag1 - C_g
    total = -cross + diag
    n_s = N_G + N_L
    n_loss_terms = N_T * n_s - min(N_T, n_s)
    loss = total / n_loss_terms
    healthy &= bool(np.isfinite(loss))
    return loss, healthy


def _numpy_loss(sg_full, sl_full, teacher, ts, tt):
    """Exact host fallback (never hit for sane input distributions)."""
    x = np.concatenate([sg_full, sl_full], axis=0).astype(np.float64) / ts
    lq = x - x.max(axis=1, keepdims=True)
    lq -= np.log(np.exp(lq).sum(axis=1, keepdims=True))
    y = teacher.astype(np.float64) / tt
    e = np.exp(y - y.max(axis=1, keepdims=True))
    p = e / e.sum(axis=1, keepdims=True)
    ce = -(p @ lq.T)
    n_t, n_s = ce.shape
    idx = np.arange(n_t)
    ce[idx, idx] = 0.0
    return ce.sum() / (n_t * n_s - min(n_t, n_s))


def kernel(out_student_global, out_student_local, out_teacher, center,
           temp_student, temp_teacher, cent_rate_m):
    out_student_global = np.asarray(out_student_global)
    out_student_local = np.asarray(out_student_local)
    out_teacher = np.asarray(out_teacher)
    center = np.asarray(center)
    ts = float(np.asarray(temp_student).reshape(-1)[0])
    tt = float(np.asarray(temp_teacher).reshape(-1)[0])

    teacher = out_teacher
    if np.any(center):
        teacher = out_teacher - center.reshape(1, -1).astype(np.float32)
    teacher = np.ascontiguousarray(teacher, dtype=np.float32)
    sg_full = np.ascontiguousarray(out_student_global, dtype=np.float32)
    sl_full = np.ascontiguousarray(out_student_local, dtype=np.float32)

    t_bf = teacher.astype(BF)
    sg_bf = sg_full.astype(BF)
    sl_bf = sl_full.astype(BF)

    # Safe exp bounds: strided-sample max + margin. Softmax ratios are
    # shift-invariant, so only overflow matters (margin << 88*temp).
    smax = max(float(sl_full.ravel()[::257].max()),
               float(sg_full.ravel()[::257].max()))
    b_s = smax + 1.0
    nbs = np.full((128, 1), -b_s / ts, np.float32)
    b_t = float(teacher.ravel()[::257].max()) + 2.0
    ntb = np.full((128, 1), -b_t / tt, np.float32)

    # Diagonal term sum_i p_i . (sg_i/ts): exact f64 on the host -- O(Nt*D)
    # is ~0.1% of the kernel's flops and removes a whole device pass.
    y = teacher.astype(np.float64) / tt
    y -= y.max(axis=1, keepdims=True)
    e = np.exp(y)
    diag1 = float(
        ((e * sg_full.astype(np.float64)).sum(axis=1) / e.sum(axis=1)).sum()
        / ts)

    nc = _get_nc(ts, tt)
    in_maps = []
    for c in range(N_CORES):
        in_maps.append({
            "sl": sl_bf[c * SL_ROWS:(c + 1) * SL_ROWS],
            "sg": sg_bf[c * SG_ROWS:(c + 1) * SG_ROWS].reshape(128, D // 4),
            "t": t_bf[c * T_ROWS:(c + 1) * T_ROWS].reshape(128, D // 4),
            "nbs": nbs,
            "ntb": ntb,
        })
    res = run_bass_kernel_spmd(nc, in_maps, core_ids=list(range(N_CORES)))
    loss, healthy = _merge(res.results, ts, tt, b_s / ts, diag1)
    if not healthy:
        loss = _numpy_loss(sg_full, sl_full, teacher, ts, tt)
    return np.float32(loss)


# revision 15
# speedup vs baseline: 2.2050x; 1.0266x over previous
"""DINO loss kernel for Trainium2 (8 NeuronCores, Bass/Tile).

Math
----
Reference computes, with q = log_softmax(student/ts) [Ns=1280, D] and
p = softmax((teacher-center)/tt) [Nt=256, D]:

    loss = sum_{i != j} ( -sum_d p[i,d] q[j,d] ) / (Nt*Ns - Nt)

The full-pair sum factorizes over d:

    sum_{i,j} ce[i,j] = -sum_d P[d] * Q[d]
      P[d] = sum_i p[i,d]                (teacher prob column sums)
      Q[d] = sum_j q[j,d] = S[d]/ts - C  (S = raw student logit column sums,
                                          C = sum_j logsumexp_j(x/ts))
    diag  = sum_i sum_d p[i,d] q_g[i,d]
          = sum_i v_i/(ts*Z_i) - C_g     (v_i = sum_d e_t[i,d]*sg[i,d])

    loss = ( -(dot(P,S)/ts - C*sum(P)) + diag ) / (Nt*Ns - Nt)

So the device only does streaming reductions (no [Nt,Ns,D] einsum):
row sum-exp stats, raw column sums, teacher-prob column sums, and the
elementwise teacher*student_global dot for the diagonal.

Sharding (8 cores)
------------------
Pure data parallel over rows, one NEFF run, no collectives:
  core c gets student_local rows [128c,128c+128)           -> sl  [128, 65536]
           student_global rows [32c,32c+32) row-split x4   -> sg  [128, 16384]
           teacher rows        [32c,32c+32) row-split x4   -> t   [128, 16384]
Row-split x4: row i of a [32, 65536] slice is spread over partitions
4i..4i+3, 16384 columns each (a plain reshape(128, 16384) on the host),
so all engines run at full 128-partition width.

Implementation notes
--------------------
* All big inputs travel as bf16 (half the HBM/DMA traffic; quantization
  error on the final scalar is ~2e-5, tolerance is 2e-2). The teacher is
  exp'd in place in bf16; p normalization (e/Z) uses consistently rounded
  values so ratios stay exact to bf16 element precision.
* Both student AND teacher exps use host-sampled upper bounds (sample max
  + margin) as the common exp bias. Softmax ratios are shift-invariant,
  so the bound only has to avoid overflow (margin keeps x - b well below
  88*tt) -- elements far below the bound underflow to 0 and contribute
  ~e^-60 relative, i.e. nothing. This removes the whole on-device teacher
  row-max fold chain. If any stat comes back non-finite (pathological
  input distribution), kernel() falls back to an exact numpy evaluation.
* Column sums run on the PE as mask-weighted bf16 matmuls (1 cyc/row).
  Bank tiles are [4, 1024] in PSUM (exactly the real output rows).
* DMA queue discipline (the big scheduling win): the SP/sync queue
  carries ONLY input loads, in consumption order, so no compute-dependent
  DMA ever stalls a load (a waiting DMA holds its engine's sequencer).
  PSUM retires for the student_local stream go PSUM->DRAM directly on the
  Pool/SWDGE queue; sg/p banks stage through SBUF on DVE (fast PSUM WAR
  release so the PE never stalls on bank reuse) and then DRAM-out on Pool.
* All cross-core / cross-partition-group merging is float64 on the host.
"""

import numpy as np
import ml_dtypes

import concourse.bass as bass
import concourse.bacc as bacc
import concourse.tile as tile
from concourse import mybir
from concourse.bass_utils import run_bass_kernel_spmd

F32 = mybir.dt.float32
BF16 = mybir.dt.bfloat16
AX = mybir.AxisListType
EXP = mybir.ActivationFunctionType.Exp
BF = ml_dtypes.bfloat16

N_CORES = 8
D = 65536
N_T = 256
N_G = 256
N_L = 1024
SL_ROWS = N_L // N_CORES          # 128 student_local rows per core
SG_ROWS = N_G // N_CORES          # 32 student_global rows per core
T_ROWS = N_T // N_CORES           # 32 teacher rows per core


def _masks(P=128):
    # qmask[p, m] = 1 if m == p % 4: out row m = colsum over partition group
    # m (row-split x4 quarter colsums, weighted later by 1/Z for P).
    qmask = np.zeros((P, 4), BF)
    qmask[np.arange(P), np.arange(P) % 4] = 1.0
    # emask block q ([:, 4q:4q+4]) has ones only in column q: lhsT that adds
    # a plain colsum of quarter q into row q of a 4-row PSUM region.
    emask = np.zeros((P, 16), BF)
    for q in range(4):
        emask[:, 4 * q + q] = 1.0
    # gmask[p', p] = 1 iff p'//4 == p//4: one matmul folds the 4 per-quarter
    # partial Z's of each logical row and broadcasts to all 4 partitions.
    gmask = np.zeros((P, P), BF)
    for r in range(P // 4):
        gmask[4 * r : 4 * r + 4, 4 * r : 4 * r + 4] = 1.0
    return qmask, emask, gmask


def build_nc(D=D, n_sl_chunks=8, ts=0.1, tt=0.04):
    """Build the per-core Bass program. All 8 cores run this same NEFF."""
    DQ = D // 4                    # 16384 columns per quarter
    CQ = DQ // n_sl_chunks         # 2048 sl quarter-cols per DMA chunk
    reg = 512                      # matmul free size (one PSUM bank of f32)
    bank_n = 2 * reg               # quarter-cols per PSUM bank tile
    assert CQ % bank_n == 0
    bpc = CQ // bank_n             # sl bank tiles per chunk
    nb = DQ // bank_n              # bank tiles per full stream (sg/p)
    cht = DQ // 4                  # teacher/sg DMA+exp chunk size

    nc = bacc.Bacc()
    sl = nc.dram_tensor("sl", [128, D], BF16, kind="ExternalInput")
    sg = nc.dram_tensor("sg", [128, DQ], BF16, kind="ExternalInput")
    t = nc.dram_tensor("t", [128, DQ], BF16, kind="ExternalInput")
    nb2 = nc.dram_tensor("nb2", [128, 2], F32, kind="ExternalInput")

    qmask_np, emask_np, gmask_np = _masks()
    masks_d = nc.inline_tensor(
        np.concatenate([qmask_np, emask_np, gmask_np], axis=1), name="masks_c")

    s_sl = nc.dram_tensor("s_sl", [4, DQ], F32, kind="ExternalOutput")
    s_sg = nc.dram_tensor("s_sg", [4, DQ], F32, kind="ExternalOutput")
    p_out = nc.dram_tensor("p_out", [4, DQ], F32, kind="ExternalOutput")
    w_sl = nc.dram_tensor("w_sl", [128, n_sl_chunks], F32, kind="ExternalOutput")
    w_sg = nc.dram_tensor("w_sg", [128, 4], F32, kind="ExternalOutput")

    with tile.TileContext(nc) as tc:
        with (
            tc.tile_pool(name="singles", bufs=1) as singles,
            tc.tile_pool(name="big", bufs=1) as big,
            tc.tile_pool(name="chunks", bufs=3) as chunks,
            tc.tile_pool(name="escr", bufs=1) as escr,
            tc.tile_pool(name="stats", bufs=1) as stats,
            tc.tile_pool(name="stage", bufs=4) as stage_pool,
            tc.tile_pool(name="psA", bufs=2, space="PSUM") as psA,
            tc.tile_pool(name="psB", bufs=3, space="PSUM") as psB,
            tc.tile_pool(name="psC", bufs=1, space="PSUM") as psC,
        ):
            # The SP/sync queue carries ONLY input loads, in consumption
            # order. Nothing on it ever waits on compute.
            tr = big.tile([128, DQ], BF16)
            sgr = big.tile([128, DQ], BF16)
            # Teacher arrives in 5 pieces -- a small first piece so the ACT
            # exp stream starts as early as possible.
            tch = [(0, 2048), (2048, 2048), (4096, 4096),
                   (8192, 4096), (12288, 4096)]
            nc.sync.dma_start(out=tr[:, 0:2048], in_=t[:, 0:2048])
            nb2_t = singles.tile([128, 2], F32)
            nc.sync.dma_start(out=nb2_t, in_=nb2[:, :])
            nbs_t = nb2_t[:, 0:1]
            ntb_t = nb2_t[:, 1:2]
            masks = singles.tile([128, 148], BF16)
            nc.sync.dma_start(out=masks, in_=masks_d[:, :])
            qmask = masks[:, 0:4]
            emask = masks[:, 4:20]
            gmask = masks[:, 20:148]
            for lo, n in tch[1:]:
                nc.sync.dma_start(out=tr[:, lo : lo + n], in_=t[:, lo : lo + n])
            for j in range(4):
                nc.sync.dma_start(
                    out=sgr[:, j * cht : (j + 1) * cht],
                    in_=sg[:, j * cht : (j + 1) * cht],
                )

            # Warm the ACT exp table while the first teacher piece loads.
            warm = stats.tile([128, 1], F32)
            nc.vector.memset(warm, 0.0)
            nc.scalar.activation(warm, warm, EXP)

            # teacher exp (in place, bf16) + per-partition partial Z sums
            zT = stats.tile([128, len(tch)], F32)
            for j, (lo, n) in enumerate(tch):
                nc.scalar.activation(
                    tr[:, lo : lo + n],
                    tr[:, lo : lo + n],
                    EXP, bias=ntb_t, scale=1.0 / tt,
                    accum_out=zT[:, j : j + 1],
                )

            # Z fold across the 4 partitions of each logical teacher row:
            # one block-diagonal matmul broadcasts the group sums back to
            # every partition (NO DMA -- a tiny compute-dependent DMA here
            # would queue behind all prefetched loads on the DMA engines).
            zloc = stats.tile([128, 1], BF16)
            with nc.allow_low_precision(reason="Z fold feeds a bf16 matmul; "
                                        "0.4% on Z is far inside tolerance"):
                nc.vector.reduce_sum(zloc, zT, axis=AX.X)
            zfold = psC.tile([128, 1], F32)
            nc.tensor.matmul(zfold, gmask, zloc, start=True, stop=True)
            rzb = stats.tile([128, 1], F32)
            nc.vector.reciprocal(rzb, zfold)
            wq = stats.tile([128, 4], BF16)
            nc.vector.tensor_scalar_mul(wq, qmask, rzb)

            # student_global exp stats (scratch out; sgr stays raw; same
            # host-supplied bound as student_local).
            wG = stats.tile([128, 4], F32)

            def sg_exp(j):
                sc = escr.tile([128, cht], BF16, tag="escr")
                nc.scalar.activation(
                    sc, sgr[:, j * cht : (j + 1) * cht],
                    EXP, bias=nbs_t, scale=1.0 / ts,
                    accum_out=wG[:, j : j + 1],
                )

            # PSUM retire: bank -> SBUF stage slot on DVE (fast WAR release
            # so the PE never waits on a DMA round-trip); one Pool/SWDGE DMA
            # stores each filled [4, 4096] stage tile. Big stage groups +
            # deep buffering so store transfers queueing behind prefetched
            # loads on the DMA engines never back-propagates to the PE.
            def make_stream(dst, bank_cols, group):
                state = {"st": None, "n0": 0, "cnt": 0}

                def add(bank, bank_i):
                    if state["st"] is None:
                        st = stage_pool.tile(
                            [4, group * bank_cols], F32, tag="stage",
                            name="st")
                        state["st"] = st
                        state["n0"] = bank_i
                        state["cnt"] = 0
                    q = state["cnt"]
                    nc.vector.tensor_copy(
                        out=state["st"][:, q * bank_cols : (q + 1) * bank_cols],
                        in_=bank)
                    state["cnt"] += 1
                    if state["cnt"] == group:
                        nc.gpsimd.dma_start(
                            out=dst[:, state["n0"] * bank_cols
                                    : (state["n0"] + group) * bank_cols],
                            in_=state["st"])
                        state["st"] = None
                return add

            sg_ret = make_stream(s_sg, bank_n, 4)
            p_ret = make_stream(p_out, bank_n, 4)
            sl_ret = make_stream(s_sl, reg, 8)

            def _abank(lhsT, srct, ret, bank_i):
                bank = psA.tile([4, bank_n], F32, tag="bankA")
                for s in range(bank_n // reg):
                    lo = bank_i * bank_n + s * reg
                    nc.tensor.matmul(
                        bank[:, s * reg : (s + 1) * reg],
                        lhsT,
                        srct[:, lo : lo + reg],
                        start=True, stop=True,
                    )
                ret(bank, bank_i)

            def sg_bank(bank_i):
                _abank(qmask, sgr, sg_ret, bank_i)

            def p_bank(bank_i):
                _abank(wq, tr, p_ret, bank_i)

            # student_local stream: chunk DMA (sync queue), colsum matmuls
            # (PE, [4, reg] banks), exp on ACT (throwaway scratch + row-sum
            # accumulator), retire via the shared stage/store path.
            slv = sl.rearrange("p (q k c) -> p q k c", q=4, k=n_sl_chunks)
            wS = stats.tile([128, n_sl_chunks], F32)

            def sl_chunk(k):
                ch = chunks.tile([128, 4, CQ], BF16, tag="chunk")
                nc.sync.dma_start(out=ch, in_=slv[:, :, k, :])
                sc = escr.tile([128, 4 * CQ], BF16, tag="escr")
                nc.scalar.activation(
                    sc.rearrange("p (q c) -> p q c", q=4), ch, EXP,
                    bias=nbs_t, scale=1.0 / ts,
                    accum_out=wS[:, k : k + 1],
                )
                for b in range(CQ // reg):
                    bank = psB.tile([4, reg], F32, tag="bankB")
                    cl = b * reg
                    for q in range(4):
                        nc.tensor.matmul(
                            bank,
                            emask[:, 4 * q : 4 * q + 4],
                            ch[:, q, cl : cl + reg],
                            start=(q == 0),
                            stop=(q == 3),
                        )
                    sl_ret(bank, k * (CQ // reg) + b)

            # ---- interleaved schedule (per-engine order == emission; the
            # tile scheduler may refine within dependency limits) ----
            for j in range(4):
                sg_exp(j)
            for i in range(nb):
                sg_bank(i)
            sl_chunk(0)
            for i in range(0, 4):
                p_bank(i)
            sl_chunk(1)
            for i in range(4, 8):
                p_bank(i)
            sl_chunk(2)
            for i in range(8, 12):
                p_bank(i)
            sl_chunk(3)
            for i in range(12, 16):
                p_bank(i)
            for k in range(4, n_sl_chunks):
                sl_chunk(k)

            # stats out: w_sg rides the Pool queue (ready mid-stream); w_sl
            # (gated by the final sl exp) goes at the very end of ACT's own
            # queue so it cannot head-of-line-block the Pool stores.
            nc.gpsimd.dma_start(out=w_sg[:, :], in_=wG)
            nc.scalar.dma_start(out=w_sl[:, :], in_=wS)

    nc.compile()
    return nc


_NC_CACHE = {}


def _get_nc(ts, tt):
    key = (round(ts, 9), round(tt, 9))
    if key not in _NC_CACHE:
        _NC_CACHE[key] = build_nc(ts=ts, tt=tt)
    return _NC_CACHE[key]


def _merge(results, ts, tt, bs_scaled, diag1, n_sl_chunks=8):
    """Host-side exact merge of per-core device outputs (float64).

    bs_scaled = b_s/ts, the (already scaled) exp bound the device used for
    student rows; diag1 = sum_i v_i/(ts*Z_i), computed exactly on the host
    (O(Nt*D), ~0.1% of the kernel flops). Returns (loss, healthy).
    """
    S = np.zeros(D, np.float64)
    P = np.zeros(D, np.float64)
    C = 0.0       # sum of all student row logsumexps
    C_g = 0.0     # global-student-row portion
    healthy = True
    for r in results:
        S += r["s_sl"].astype(np.float64).reshape(-1)
        S += r["s_sg"].astype(np.float64).reshape(-1)
        P += r["p_out"].astype(np.float64).reshape(-1)
        # student_local rows: common bound -> lse = b/ts + log(sum w)
        w = r["w_sl"].astype(np.float64)               # [128, nch]
        wsum = w.sum(axis=1)
        healthy &= bool(np.isfinite(w).all() and (wsum > 0).all())
        C += (bs_scaled + np.log(np.maximum(wsum, 1e-300))).sum()
        # student_global rows: common bound per-partition lse -> merge 4s
        wg = r["w_sg"].astype(np.float64).sum(axis=1)  # [128]
        healthy &= bool(np.isfinite(wg).all() and (wg > 0).all())
        lp = (bs_scaled + np.log(np.maximum(wg, 1e-300))).reshape(32, 4)
        mxg = lp.max(axis=1, keepdims=True)
        lse_g = mxg[:, 0] + np.log(np.exp(lp - mxg).sum(axis=1))
        C += lse_g.sum()
        C_g += lse_g.sum()
        healthy &= bool(np.isfinite(r["s_sl"]).all()
                        and np.isfinite(r["s_sg"]).all()
                        and np.isfinite(r["p_out"]).all())

    cross = P @ S / ts - C * P.sum()
    diag = diag1 - C_g
    total = -cross + diag
    n_s = N_G + N_L
    n_loss_terms = N_T * n_s - min(N_T, n_s)
    loss = total / n_loss_terms
    healthy &= bool(np.isfinite(loss))
    return loss, healthy


def _numpy_loss(sg_full, sl_full, teacher, ts, tt):
    """Exact host fallback (never hit for sane input distributions)."""
    x = np.concatenate([sg_full, sl_full], axis=0).astype(np.float64) / ts
    lq = x - x.max(axis=1, keepdims=True)
    lq -= np.log(np.exp(lq).sum(axis=1, keepdims=True))
    y = teacher.astype(np.float64) / tt
    e = np.exp(y - y.max(axis=1, keepdims=True))
    p = e / e.sum(axis=1, keepdims=True)
    ce = -(p @ lq.T)
    n_t, n_s = ce.shape
    idx = np.arange(n_t)
    ce[idx, idx] = 0.0
    return ce.sum() / (n_t * n_s - min(n_t, n_s))


def kernel(out_student_global, out_student_local, out_teacher, center,
           temp_student, temp_teacher, cent_rate_m):
    out_student_global = np.asarray(out_student_global)
    out_student_local = np.asarray(out_student_local)
    out_teacher = np.asarray(out_teacher)
    center = np.asarray(center)
    ts = float(np.asarray(temp_student).reshape(-1)[0])
    tt = float(np.asarray(temp_teacher).reshape(-1)[0])

    teacher = out_teacher
    if np.any(center):
        teacher = out_teacher - center.reshape(1, -1).astype(np.float32)
    teacher = np.ascontiguousarray(teacher, dtype=np.float32)
    sg_full = np.ascontiguousarray(out_student_global, dtype=np.float32)
    sl_full = np.ascontiguousarray(out_student_local, dtype=np.float32)

    t_bf = teacher.astype(BF)
    sg_bf = sg_full.astype(BF)
    sl_bf = sl_full.astype(BF)

    # Safe exp bounds: strided-sample max + margin. Softmax ratios are
    # shift-invariant, so only overflow matters (margin << 88*temp).
    smax = max(float(sl_full.ravel()[::257].max()),
               float(sg_full.ravel()[::257].max()))
    b_s = smax + 1.0
    b_t = float(teacher.ravel()[::257].max()) + 2.0
    nb2 = np.empty((128, 2), np.float32)
    nb2[:, 0] = -b_s / ts
    nb2[:, 1] = -b_t / tt

    # Diagonal term sum_i p_i . (sg_i/ts): exact f64 on the host -- O(Nt*D)
    # is ~0.1% of the kernel's flops and removes a whole device pass.
    y = teacher.astype(np.float64) / tt
    y -= y.max(axis=1, keepdims=True)
    e = np.exp(y)
    diag1 = float(
        ((e * sg_full.astype(np.float64)).sum(axis=1) / e.sum(axis=1)).sum()
        / ts)

    nc = _get_nc(ts, tt)
    in_maps = []
    for c in range(N_CORES):
        in_maps.append({
            "sl": sl_bf[c * SL_ROWS:(c + 1) * SL_ROWS],
            "sg": sg_bf[c * SG_ROWS:(c + 1) * SG_ROWS].reshape(128, D // 4),
            "t": t_bf[c * T_ROWS:(c + 1) * T_ROWS].reshape(128, D // 4),
            "nb2": nb2,
        })
    res = run_bass_kernel_spmd(nc, in_maps, core_ids=list(range(N_CORES)))
    loss, healthy = _merge(res.results, ts, tt, b_s / ts, diag1)
    if not healthy:
        loss = _numpy_loss(sg_full, sl_full, teacher, ts, tt)
    return np.float32(loss)


# revision 27
# speedup vs baseline: 2.2391x; 1.0154x over previous
"""DINO loss kernel for Trainium2 (8 NeuronCores, Bass/Tile).

Math
----
Reference computes, with q = log_softmax(student/ts) [Ns=1280, D] and
p = softmax((teacher-center)/tt) [Nt=256, D]:

    loss = sum_{i != j} ( -sum_d p[i,d] q[j,d] ) / (Nt*Ns - Nt)

The full-pair sum factorizes over d:

    sum_{i,j} ce[i,j] = -sum_d P[d] * Q[d]
      P[d] = sum_i p[i,d]                (teacher prob column sums)
      Q[d] = sum_j q[j,d] = S[d]/ts - C  (S = raw student logit column sums,
                                          C = sum_j logsumexp_j(x/ts))
    diag  = sum_i sum_d p[i,d] q_g[i,d]
          = sum_i v_i/(ts*Z_i) - C_g     (v_i = sum_d e_t[i,d]*sg[i,d])

    loss = ( -(dot(P,S)/ts - C*sum(P)) + diag ) / (Nt*Ns - Nt)

So the device only does streaming reductions (no [Nt,Ns,D] einsum):
row sum-exp stats, raw column sums, teacher-prob column sums, and the
elementwise teacher*student_global dot for the diagonal.

Sharding (8 cores)
------------------
Pure data parallel over rows, one NEFF run, no collectives:
  core c gets student_local rows [128c,128c+128)           -> sl  [128, 65536]
           student_global rows [32c,32c+32) row-split x4   -> sg  [128, 16384]
           teacher rows        [32c,32c+32) row-split x4   -> t   [128, 16384]
Row-split x4: row i of a [32, 65536] slice is spread over partitions
4i..4i+3, 16384 columns each (a plain reshape(128, 16384) on the host),
so all engines run at full 128-partition width.

Implementation notes
--------------------
* All big inputs travel as bf16 (half the HBM/DMA traffic; quantization
  error on the final scalar is ~2e-5, tolerance is 2e-2). The teacher is
  exp'd in place in bf16; p normalization (e/Z) uses consistently rounded
  values so ratios stay exact to bf16 element precision.
* Both student AND teacher exps use host-sampled upper bounds (sample max
  + margin) as the common exp bias. Softmax ratios are shift-invariant,
  so the bound only has to avoid overflow (margin keeps x - b well below
  88*tt) -- elements far below the bound underflow to 0 and contribute
  ~e^-60 relative, i.e. nothing. This removes the whole on-device teacher
  row-max fold chain. If any stat comes back non-finite (pathological
  input distribution), kernel() falls back to an exact numpy evaluation.
* Column sums run on the PE as mask-weighted bf16 matmuls (1 cyc/row).
  Bank tiles are [4, 1024] in PSUM (exactly the real output rows).
* DMA queue discipline (the big scheduling win): the SP/sync queue
  carries ONLY input loads, in consumption order, so no compute-dependent
  DMA ever stalls a load (a waiting DMA holds its engine's sequencer).
  PSUM retires for the student_local stream go PSUM->DRAM directly on the
  Pool/SWDGE queue; sg/p banks stage through SBUF on DVE (fast PSUM WAR
  release so the PE never stalls on bank reuse) and then DRAM-out on Pool.
* All cross-core / cross-partition-group merging is float64 on the host.
"""

import numpy as np
import ml_dtypes

import concourse.bass as bass
import concourse.bacc as bacc
import concourse.tile as tile
from concourse import mybir
from concourse.bass_utils import run_bass_kernel_spmd

F32 = mybir.dt.float32
BF16 = mybir.dt.bfloat16
AX = mybir.AxisListType
EXP = mybir.ActivationFunctionType.Exp
BF = ml_dtypes.bfloat16

N_CORES = 8
D = 65536
N_T = 256
N_G = 256
N_L = 1024
SL_ROWS = N_L // N_CORES          # 128 student_local rows per core
SG_ROWS = N_G // N_CORES          # 32 student_global rows per core
T_ROWS = N_T // N_CORES           # 32 teacher rows per core


def _masks(P=128):
    # qmask[p, m] = 1 if m == p % 4: out row m = colsum over partition group
    # m (row-split x4 quarter colsums, weighted later by 1/Z for P).
    qmask = np.zeros((P, 4), BF)
    qmask[np.arange(P), np.arange(P) % 4] = 1.0
    # emask block q ([:, 4q:4q+4]) has ones only in column q: lhsT that adds
    # a plain colsum of quarter q into row q of a 4-row PSUM region.
    emask = np.zeros((P, 16), BF)
    for q in range(4):
        emask[:, 4 * q + q] = 1.0
    # gmask[p', p] = 1 iff p'//4 == p//4: one matmul folds the 4 per-quarter
    # partial Z's of each logical row and broadcasts to all 4 partitions.
    gmask = np.zeros((P, P), BF)
    for r in range(P // 4):
        gmask[4 * r : 4 * r + 4, 4 * r : 4 * r + 4] = 1.0
    return qmask, emask, gmask


def build_nc(D=D, n_sl_chunks=8, ts=0.1, tt=0.04):
    """Build the per-core Bass program. All 8 cores run this same NEFF."""
    DQ = D // 4                    # 16384 columns per quarter
    CQ = DQ // n_sl_chunks         # 2048 sl quarter-cols per DMA chunk
    reg = 512                      # matmul free size (one PSUM bank of f32)
    bank_n = 2 * reg               # quarter-cols per PSUM bank tile
    assert CQ % bank_n == 0
    bpc = CQ // bank_n             # sl bank tiles per chunk
    nb = DQ // bank_n              # bank tiles per full stream (sg/p)
    cht = DQ // 4                  # teacher/sg DMA+exp chunk size

    nc = bacc.Bacc()
    sl = nc.dram_tensor("sl", [128, D], BF16, kind="ExternalInput")
    sg = nc.dram_tensor("sg", [128, DQ], BF16, kind="ExternalInput")
    t = nc.dram_tensor("t", [128, DQ], BF16, kind="ExternalInput")
    nb2 = nc.dram_tensor("nb2", [128, 2], F32, kind="ExternalInput")

    qmask_np, emask_np, gmask_np = _masks()
    masks_d = nc.inline_tensor(
        np.concatenate([qmask_np, emask_np, gmask_np], axis=1), name="masks_c")

    s_sl = nc.dram_tensor("s_sl", [4, DQ], BF16, kind="ExternalOutput")
    s_sg = nc.dram_tensor("s_sg", [4, DQ], BF16, kind="ExternalOutput")
    p_out = nc.dram_tensor("p_out", [4, DQ], BF16, kind="ExternalOutput")
    w_sl = nc.dram_tensor("w_sl", [128, n_sl_chunks], F32, kind="ExternalOutput")
    w_sg = nc.dram_tensor("w_sg", [128, 4], F32, kind="ExternalOutput")

    with tile.TileContext(nc) as tc:
        with (
            tc.tile_pool(name="singles", bufs=1) as singles,
            tc.tile_pool(name="big", bufs=1) as big,
            tc.tile_pool(name="chunks", bufs=3) as chunks,
            tc.tile_pool(name="escr", bufs=1) as escr,
            tc.tile_pool(name="stats", bufs=1) as stats,
            tc.tile_pool(name="stage", bufs=6) as stage_pool,
            tc.tile_pool(name="psA", bufs=2, space="PSUM") as psA,
            tc.tile_pool(name="psB", bufs=3, space="PSUM") as psB,
            tc.tile_pool(name="psC", bufs=1, space="PSUM") as psC,
        ):
            # The SP/sync queue carries ONLY input loads, in consumption
            # order. Nothing on it ever waits on compute.
            tr = big.tile([128, DQ], BF16)
            sgr = big.tile([128, DQ], BF16)
            # Teacher arrives in 5 pieces -- a small first piece so the ACT
            # exp stream starts as early as possible.
            tch = [(0, 2048), (2048, 2048), (4096, 4096),
                   (8192, 4096), (12288, 4096)]
            nc.sync.dma_start(out=tr[:, 0:2048], in_=t[:, 0:2048])
            nb2_t = singles.tile([128, 2], F32)
            nc.sync.dma_start(out=nb2_t, in_=nb2[:, :])
            nbs_t = nb2_t[:, 0:1]
            ntb_t = nb2_t[:, 1:2]
            lo, n = tch[1]
            nc.sync.dma_start(out=tr[:, lo : lo + n], in_=t[:, lo : lo + n])
            masks = singles.tile([128, 148], BF16)
            nc.sync.dma_start(out=masks, in_=masks_d[:, :])
            qmask = masks[:, 0:4]
            emask = masks[:, 4:20]
            gmask = masks[:, 20:148]
            for lo, n in tch[2:]:
                nc.sync.dma_start(out=tr[:, lo : lo + n], in_=t[:, lo : lo + n])
            for j in range(4):
                nc.sync.dma_start(
                    out=sgr[:, j * cht : (j + 1) * cht],
                    in_=sg[:, j * cht : (j + 1) * cht],
                )

            # Warm the ACT exp table while the first teacher piece loads.
            warm = stats.tile([128, 1], F32)
            nc.vector.memset(warm, 0.0)
            nc.scalar.activation(warm, warm, EXP)

            # teacher exp (in place, bf16) + per-partition partial Z sums
            zT = stats.tile([128, len(tch)], F32)
            for j, (lo, n) in enumerate(tch):
                nc.scalar.activation(
                    tr[:, lo : lo + n],
                    tr[:, lo : lo + n],
                    EXP, bias=ntb_t, scale=1.0 / tt,
                    accum_out=zT[:, j : j + 1],
                )

            # Z fold across the 4 partitions of each logical teacher row:
            # one block-diagonal matmul broadcasts the group sums back to
            # every partition (NO DMA -- a tiny compute-dependent DMA here
            # would queue behind all prefetched loads on the DMA engines).
            zloc = stats.tile([128, 1], BF16)
            with nc.allow_low_precision(reason="Z fold feeds a bf16 matmul; "
                                        "0.4% on Z is far inside tolerance"):
                nc.vector.reduce_sum(zloc, zT, axis=AX.X)
            zfold = psC.tile([128, 1], F32)
            nc.tensor.matmul(zfold, gmask, zloc, start=True, stop=True)
            rzb = stats.tile([128, 1], F32)
            nc.vector.reciprocal(rzb, zfold)
            wq = stats.tile([128, 4], BF16)
            nc.vector.tensor_scalar_mul(wq, qmask, rzb)

            # student_global exp stats (scratch out; sgr stays raw; same
            # host-supplied bound as student_local).
            wG = stats.tile([128, 4], F32)

            def sg_exp(j):
                sc = escr.tile([128, cht], BF16, tag="escr")
                nc.scalar.activation(
                    sc, sgr[:, j * cht : (j + 1) * cht],
                    EXP, bias=nbs_t, scale=1.0 / ts,
                    accum_out=wG[:, j : j + 1],
                )

            # PSUM retire: bank -> SBUF stage slot on DVE (fast WAR release
            # so the PE never waits on a DMA round-trip); one Pool/SWDGE DMA
            # stores each filled [4, 4096] stage tile. Big stage groups +
            # deep buffering so store transfers queueing behind prefetched
            # loads on the DMA engines never back-propagates to the PE.
            def make_stream(dst, bank_cols, group):
                state = {"st": None, "n0": 0, "cnt": 0}

                def add(bank, bank_i):
                    if state["st"] is None:
                        st = stage_pool.tile(
                            [4, group * bank_cols], BF16, tag="stage",
                            name="st")
                        state["st"] = st
                        state["n0"] = bank_i
                        state["cnt"] = 0
                    q = state["cnt"]
                    with nc.allow_low_precision(
                            reason="bf16 colsum outputs; 0.4% per column is "
                            "far inside the 2e-2 gate"):
                        nc.vector.tensor_copy(
                            out=state["st"][:, q * bank_cols
                                            : (q + 1) * bank_cols],
                            in_=bank)
                    state["cnt"] += 1
                    if state["cnt"] == group:
                        nc.gpsimd.dma_start(
                            out=dst[:, state["n0"] * bank_cols
                                    : (state["n0"] + group) * bank_cols],
                            in_=state["st"])
                        state["st"] = None
                return add

            sg_ret = make_stream(s_sg, bank_n, 4)
            p_ret = make_stream(p_out, bank_n, 4)
            sl_ret = make_stream(s_sl, reg, 8)

            def _abank(lhsT, srct, ret, bank_i):
                bank = psA.tile([4, bank_n], F32, tag="bankA")
                for s in range(bank_n // reg):
                    lo = bank_i * bank_n + s * reg
                    nc.tensor.matmul(
                        bank[:, s * reg : (s + 1) * reg],
                        lhsT,
                        srct[:, lo : lo + reg],
                        start=True, stop=True,
                    )
                ret(bank, bank_i)

            def sg_bank(bank_i):
                _abank(qmask, sgr, sg_ret, bank_i)

            def p_bank(bank_i):
                _abank(wq, tr, p_ret, bank_i)

            # student_local stream: chunk DMA (sync queue), colsum matmuls
            # (PE, [4, reg] banks), exp on ACT (throwaway scratch + row-sum
            # accumulator), retire via the shared stage/store path.
            slv = sl.rearrange("p (q k c) -> p q k c", q=4, k=n_sl_chunks)
            wS = stats.tile([128, n_sl_chunks], F32)

            def sl_chunk(k):
                ch = chunks.tile([128, 4, CQ], BF16, tag="chunk")
                nc.sync.dma_start(out=ch, in_=slv[:, :, k, :])
                sc = escr.tile([128, 4 * CQ], BF16, tag="escr")
                nc.scalar.activation(
                    sc.rearrange("p (q c) -> p q c", q=4), ch, EXP,
                    bias=nbs_t, scale=1.0 / ts,
                    accum_out=wS[:, k : k + 1],
                )
                for b in range(CQ // reg):
                    bank = psB.tile([4, reg], F32, tag="bankB")
                    cl = b * reg
                    for q in range(4):
                        nc.tensor.matmul(
                            bank,
                            emask[:, 4 * q : 4 * q + 4],
                            ch[:, q, cl : cl + reg],
                            start=(q == 0),
                            stop=(q == 3),
                        )
                    sl_ret(bank, k * (CQ // reg) + b)

            # ---- interleaved schedule (per-engine order == emission; the
            # tile scheduler may refine within dependency limits) ----
            for j in range(4):
                sg_exp(j)
            for i in range(nb):
                sg_bank(i)
            sl_chunk(0)
            for i in range(0, 4):
                p_bank(i)
            sl_chunk(1)
            for i in range(4, 8):
                p_bank(i)
            sl_chunk(2)
            for i in range(8, 12):
                p_bank(i)
            sl_chunk(3)
            for i in range(12, 16):
                p_bank(i)
            for k in range(4, n_sl_chunks):
                sl_chunk(k)

            # stats out: w_sg rides the Pool queue (ready mid-stream); w_sl
            # (gated by the final sl exp) goes at the very end of ACT's own
            # queue so it cannot head-of-line-block the Pool stores.
            nc.gpsimd.dma_start(out=w_sg[:, :], in_=wG)
            nc.scalar.dma_start(out=w_sl[:, :], in_=wS)

    nc.compile()
    return nc


_NC_CACHE = {}


def _get_nc(ts, tt):
    key = (round(ts, 9), round(tt, 9))
    if key not in _NC_CACHE:
        _NC_CACHE[key] = build_nc(ts=ts, tt=tt)
    return _NC_CACHE[key]


def _merge(results, ts, tt, bs_scaled, diag1, n_sl_chunks=8):
    """Host-side exact merge of per-core device outputs (float64).

    bs_scaled = b_s/ts, the (already scaled) exp bound the device used for
    student rows; diag1 = sum_i v_i/(ts*Z_i), computed exactly on the host
    (O(Nt*D), ~0.1% of the kernel flops). Returns (loss, healthy).
    """
    S = np.zeros(D, np.float64)
    P = np.zeros(D, np.float64)
    C = 0.0       # sum of all student row logsumexps
    C_g = 0.0     # global-student-row portion
    healthy = True
    for r in results:
        S += r["s_sl"].astype(np.float64).reshape(-1)
        S += r["s_sg"].astype(np.float64).reshape(-1)
        P += r["p_out"].astype(np.float64).reshape(-1)
        # student_local rows: common bound -> lse = b/ts + log(sum w)
        w = r["w_sl"].astype(np.float64)               # [128, nch]
        wsum = w.sum(axis=1)
        healthy &= bool(np.isfinite(w).all() and (wsum > 0).all())
        C += (bs_scaled + np.log(np.maximum(wsum, 1e-300))).sum()
        # student_global rows: common bound per-partition lse -> merge 4s
        wg = r["w_sg"].astype(np.float64).sum(axis=1)  # [128]
        healthy &= bool(np.isfinite(wg).all() and (wg > 0).all())
        lp = (bs_scaled + np.log(np.maximum(wg, 1e-300))).reshape(32, 4)
        mxg = lp.max(axis=1, keepdims=True)
        lse_g = mxg[:, 0] + np.log(np.exp(lp - mxg).sum(axis=1))
        C += lse_g.sum()
        C_g += lse_g.sum()
        healthy &= bool(np.isfinite(r["s_sl"]).all()
                        and np.isfinite(r["s_sg"]).all()
                        and np.isfinite(r["p_out"]).all())

    cross = P @ S / ts - C * P.sum()
    diag = diag1 - C_g
    total = -cross + diag
    n_s = N_G + N_L
    n_loss_terms = N_T * n_s - min(N_T, n_s)
    loss = total / n_loss_terms
    healthy &= bool(np.isfinite(loss))
    return loss, healthy


def _numpy_loss(sg_full, sl_full, teacher, ts, tt):
    """Exact host fallback (never hit for sane input distributions)."""
    x = np.concatenate([sg_full, sl_full], axis=0).astype(np.float64) / ts
    lq = x - x.max(axis=1, keepdims=True)
    lq -= np.log(np.exp(lq).sum(axis=1, keepdims=True))
    y = teacher.astype(np.float64) / tt
    e = np.exp(y - y.max(axis=1, keepdims=True))
    p = e / e.sum(axis=1, keepdims=True)
    ce = -(p @ lq.T)
    n_t, n_s = ce.shape
    idx = np.arange(n_t)
    ce[idx, idx] = 0.0
    return ce.sum() / (n_t * n_s - min(n_t, n_s))


def kernel(out_student_global, out_student_local, out_teacher, center,
           temp_student, temp_teacher, cent_rate_m):
    out_student_global = np.asarray(out_student_global)
    out_student_local = np.asarray(out_student_local)
    out_teacher = np.asarray(out_teacher)
    center = np.asarray(center)
    ts = float(np.asarray(temp_student).reshape(-1)[0])
    tt = float(np.asarray(temp_teacher).reshape(-1)[0])

    teacher = out_teacher
    if np.any(center):
        teacher = out_teacher - center.reshape(1, -1).astype(np.float32)
    teacher = np.ascontiguousarray(teacher, dtype=np.float32)
    sg_full = np.ascontiguousarray(out_student_global, dtype=np.float32)
    sl_full = np.ascontiguousarray(out_student_local, dtype=np.float32)

    t_bf = teacher.astype(BF)
    sg_bf = sg_full.astype(BF)
    sl_bf = sl_full.astype(BF)

    # Safe exp bounds: strided-sample max + margin. Softmax ratios are
    # shift-invariant, so only overflow matters (margin << 88*temp).
    smax = max(float(sl_full.ravel()[::257].max()),
               float(sg_full.ravel()[::257].max()))
    b_s = smax + 1.0
    b_t = float(teacher.ravel()[::257].max()) + 2.0
    nb2 = np.empty((128, 2), np.float32)
    nb2[:, 0] = -b_s / ts
    nb2[:, 1] = -b_t / tt

    # Diagonal term sum_i p_i . (sg_i/ts): exact f64 on the host -- O(Nt*D)
    # is ~0.1% of the kernel's flops and removes a whole device pass.
    y = teacher.astype(np.float64) / tt
    y -= y.max(axis=1, keepdims=True)
    e = np.exp(y)
    diag1 = float(
        ((e * sg_full.astype(np.float64)).sum(axis=1) / e.sum(axis=1)).sum()
        / ts)

    nc = _get_nc(ts, tt)
    in_maps = []
    for c in range(N_CORES):
        in_maps.append({
            "sl": sl_bf[c * SL_ROWS:(c + 1) * SL_ROWS],
            "sg": sg_bf[c * SG_ROWS:(c + 1) * SG_ROWS].reshape(128, D // 4),
            "t": t_bf[c * T_ROWS:(c + 1) * T_ROWS].reshape(128, D // 4),
            "nb2": nb2,
        })
    res = run_bass_kernel_spmd(nc, in_maps, core_ids=list(range(N_CORES)))
    loss, healthy = _merge(res.results, ts, tt, b_s / ts, diag1)
    if not healthy:
        loss = _numpy_loss(sg_full, sl_full, teacher, ts, tt)
    return np.float32(loss)
